# revision 16
# baseline (speedup 1.0000x reference)
"""Trainium2 Bass kernel for nn_CrossAttentionViT.

Model (B=16384, S=2048, F=1024, D=1024, H=8):
  q  = spatial @ Wps.T + bps + pos_s                       (B, D)
  kv = freq @ Wpf.T + bpf + pos_f                          (B, D)
  cross-attn has seq-len 1 => softmax == 1 identically =>
      attn_out = kv @ (Wo@Wv).T + (Wo@bv + bo)
  q1 = LN1(q + attn_out)
  h  = gelu_exact(q1 @ W1.T + b1) @ W2.T + b2
  q2 = LN2(q1 + h)
  2-token self-attention over pair (q2, kv); only token 0 reaches the
  classifier => only token-0 q-projection needed; 2-way softmax ==
  sigmoid of scaled score difference.
  out = LN3(q2 + sa_out0) @ Wc.T + bc                      (B, 1)

Mapping: pure data-parallel over 8 cores (2048 rows each), processed in 4
chunks of 512 rows. Matmuls run in float32r (fp32 with 11-bit mantissa,
weights pre-rounded host-side), fp32 PSUM accumulation. Activations are
row-major for LN/softmax stages and PE-transposed to feature-major for
matmul contractions.
"""
import numpy as np
from contextlib import ExitStack

B, S_DIM, F_DIM, D, H = 16384, 2048, 1024, 1024, 8
EPS = 1e-5
NCORES = 8
R = B // NCORES            # 2048 rows per core
CHUNK = 512                # rows per chunk
NCH = R // CHUNK           # 4 chunks
NM = CHUNK // 128          # 4 row-subtiles per chunk
P = 128
SC = S_DIM // P            # 16 s-dim chunks
DC = D // P                # 8 d-dim chunks
HC = 2 * D // P            # 16 hidden chunks (2048)
NH = 2                     # two 512-wide halves of D
INV_SQRT_HD = float(1.0 / np.sqrt(D // H))

F32R_SHIFT = 12            # fp32 mantissa 23 -> 11 bits


def round_f32r(x):
    """Round fp32 array to float32r precision (11-bit mantissa, RNE)."""
    x = np.ascontiguousarray(x, dtype=np.float32)
    b = x.view(np.uint32).astype(np.uint64)
    sh = np.uint64(F32R_SHIFT)
    add = (np.uint64(1) << (sh - np.uint64(1))) - np.uint64(1)
    lsb = (b >> sh) & np.uint64(1)
    r = ((b + add + lsb) >> sh) << sh
    return (r & np.uint64(0xFFFFFFFF)).astype(np.uint32).view(np.float32).reshape(x.shape)


def _np(x):
    return np.ascontiguousarray(np.asarray(x), dtype=np.float32)


def prep_consts(inp):
    """Host-side weight folding. Returns dict of named arrays (per-core replicated)."""
    f64 = np.float64
    Wps, bps = _np(inp["Wps"]), _np(inp["bps"])
    Wpf, bpf = _np(inp["Wpf"]), _np(inp["bpf"])
    pos_s, pos_f = _np(inp["pos_s"]).reshape(-1), _np(inp["pos_f"]).reshape(-1)
    ca_Wv, ca_bv = _np(inp["ca_Wv"]), _np(inp["ca_bv"])
    ca_Wo, ca_bo = _np(inp["ca_Wo"]), _np(inp["ca_bo"])
    n1_g, n1_b = _np(inp["n1_g"]), _np(inp["n1_b"])
    W1, b1m = _np(inp["mlp_W1"]), _np(inp["mlp_b1"])
    W2, b2m = _np(inp["mlp_W2"]), _np(inp["mlp_b2"])
    n2_g, n2_b = _np(inp["n2_g"]), _np(inp["n2_b"])
    Wq, bq = _np(inp["sa_Wq"]), _np(inp["sa_bq"])
    Wk, bk = _np(inp["sa_Wk"]), _np(inp["sa_bk"])
    Wv, bv = _np(inp["sa_Wv"]), _np(inp["sa_bv"])
    Wo, bo = _np(inp["sa_Wo"]), _np(inp["sa_bo"])
    n3_g, n3_b = _np(inp["n3_g"]), _np(inp["n3_b"])
    Wc, bc = _np(inp["Wc"]), _np(inp["bc"])

    Wca = (ca_Wo.astype(f64) @ ca_Wv.astype(f64))            # (D, D) "out,in"
    b_ca = ca_Wo.astype(f64) @ ca_bv.astype(f64) + ca_bo

    c = {}
    c["wps_t"] = round_f32r(Wps.T)                            # (S, D)
    c["wpf_t"] = round_f32r(Wpf.T)                            # (F, D)
    c["wca_t"] = round_f32r(Wca.T.astype(np.float32))         # (D, D)
    c["w1_t"] = round_f32r(W1.T)                              # (D, 2D)
    c["w2_t"] = round_f32r(W2.T)                              # (2D, D)
    c["wq_t"] = round_f32r(Wq.T)
    c["wk_t"] = round_f32r(Wk.T)
    c["wv_t"] = round_f32r(Wv.T)
    c["wo_t"] = round_f32r(Wo.T)
    # stage b/c combined rank-1 bias: bps + pos_s + b_ca
    c["c_qca"] = round_f32r(((bps + pos_s).astype(f64) + b_ca).astype(np.float32)).reshape(1, D)
    # stage a (x_kv) bias, feature-major per-partition layout [128, DC]
    c["c_kv_col"] = _np(bpf + pos_f).reshape(DC, P).T.copy()          # [128, 8]
    # stage e bias: mlp_b1 + W1 @ n1_b, feature-major [128, HC]
    b_e = b1m.astype(f64) + W1.astype(f64) @ n1_b.astype(f64)
    c["b_e_col"] = b_e.astype(np.float32).reshape(HC, P).T.copy()     # [128, 16]
    # stage f rank-1 bias: n1_b + mlp_b2
    c["b_f"] = round_f32r(n1_b + b2m).reshape(1, D)
    # SA projection biases (token0 carries folded LN2 beta)
    c["bq0"] = round_f32r((Wq.astype(f64) @ n2_b.astype(f64) + bq).astype(np.float32)).reshape(1, D)
    c["bk0"] = round_f32r((Wk.astype(f64) @ n2_b.astype(f64) + bk).astype(np.float32)).reshape(1, D)
    c["bk1"] = round_f32r(bk).reshape(1, D)
    c["bv0"] = round_f32r((Wv.astype(f64) @ n2_b.astype(f64) + bv).astype(np.float32)).reshape(1, D)
    c["bv1"] = round_f32r(bv).reshape(1, D)
    # Wo stage rank-1 bias: n2_b + sa_bo
    c["b_g"] = round_f32r(n2_b + bo).reshape(1, D)
    # LN3 folded classifier row and bias
    c["wc_row"] = _np(Wc[0] * n3_g).reshape(1, D)
    c["bc_eff"] = np.array([[float(Wc[0].astype(f64) @ n3_b.astype(f64) + bc[0])]],
                           dtype=np.float32)
    c["g1_row"] = n1_g.reshape(1, D).copy()
    c["g2_row"] = n2_g.reshape(1, D).copy()
    # constants
    ident = np.eye(P, dtype=np.float32)
    c["ident32"] = ident
    c["identr"] = ident.copy()                                # exact in f32r
    ew = np.zeros((P, 1024), dtype=np.float32)
    ew[:, 512:640] = ident                                    # EW[k, 512+k] = 1
    c["ew"] = ew
    c["ones_r"] = np.ones((1, P), dtype=np.float32)
    c["bias_stack"] = np.concatenate([
        c.pop("c_qca"), c.pop("b_f"), c.pop("bq0"), c.pop("bk0"),
        c.pop("bk1"), c.pop("bv0"), c.pop("bv1"), c.pop("b_g")], axis=0)
    return c


# tensors declared float32r in DRAM (bit-compatible with fp32; pre-rounded)
_F32R_NAMES = {"spatial", "freq", "wps_t", "wpf_t", "wca_t", "w1_t", "w2_t", "wq_t", "wk_t", "wv_t",
               "wo_t", "bias_stack", "identr", "ew", "ones_r"}

_SHAPES = {
    "spatial": (R, S_DIM), "freq": (R, F_DIM),  # pre-rounded f32r
    "wps_t": (S_DIM, D), "wpf_t": (F_DIM, D), "wca_t": (D, D),
    "w1_t": (D, 2 * D), "w2_t": (2 * D, D),
    "wq_t": (D, D), "wk_t": (D, D), "wv_t": (D, D), "wo_t": (D, D),
    "bias_stack": (8, D),
    "c_kv_col": (P, DC), "b_e_col": (P, HC),
    "wc_row": (1, D), "g1_row": (1, D), "g2_row": (1, D),
    "bc_eff": (1, 1),
    "ident32": (P, P), "identr": (P, P), "ew": (P, 1024), "ones_r": (1, P),
}


def build_kernel(loop_reps=1, bias_nz=("c_qca", "b_f", "bq0", "bk0", "bk1",
                                         "bv0", "bv1", "b_g")):
    """Emit + compile the per-core Tile kernel. Returns the compiled Bass object.

    bias_nz: names of folded bias vectors that are actually nonzero; rank-1
    bias matmuls are emitted only for these."""
    import concourse.tile as tile
    import concourse.mybir as mybir
    from concourse import bacc

    dt = mybir.dt
    Alu = mybir.AluOpType
    Act = mybir.ActivationFunctionType

    nc = bacc.Bacc("TRN2", target_bir_lowering=False, debug=False)

    dram = {}
    for name, shp in _SHAPES.items():
        dty = dt.float32r if name in _F32R_NAMES else dt.float32
        dram[name] = nc.dram_tensor(name, list(shp), dty, kind="ExternalInput")
    out_d = nc.dram_tensor("out", [R, 1], dt.float32, kind="ExternalOutput")

    with tile.TileContext(nc) as tc, ExitStack() as ctx:
        cpool = ctx.enter_context(tc.tile_pool(name="consts", bufs=1))
        apool = ctx.enter_context(tc.tile_pool(name="acts", bufs=1))
        spool = ctx.enter_context(tc.tile_pool(name="small", bufs=4))
        pspool = ctx.enter_context(tc.tile_pool(name="psp", bufs=8, space="PSUM"))

        psum_ctr = [0]

        def psum(nm):
            psum_ctr[0] += 1
            return pspool.tile([P, 512], dt.float32, tag="ps",
                               name=f"{nm}_{psum_ctr[0]}")

        def cload(name, shape=None, dtype=None, src_ap=None):
            shape = list(shape or _SHAPES[name])
            dtype = dtype or (dt.float32r if name in _F32R_NAMES else dt.float32)
            t = cpool.tile(shape, dtype, name=f"c_{name}")
            nc.sync.dma_start(t[:], src_ap if src_ap is not None else dram[name].ap())
            return t

        ident32 = cload("ident32")
        identr = cload("identr")
        ew = cload("ew")
        ones_r = cload("ones_r")
        ckv_col = cload("c_kv_col")
        be_col = cload("b_e_col")
        _border = ["c_qca", "b_f", "bq0", "bk0", "bk1", "bv0", "bv1", "b_g"]
        _bidx = {k: i for i, k in enumerate(_border)}
        _bctr = [0]

        def bias_tile(key):
            """Load one bias row [1, D] from the DRAM stack into a small tile."""
            _bctr[0] += 1
            bt = apool.tile([1, D], dt.float32r, tag="mb4k", bufs=5,
                            name=f"bv_{key}_{_bctr[0]}")
            nc.sync.dma_start(
                bt[:], dram["bias_stack"].ap()[_bidx[key]:_bidx[key] + 1, :])
            return bt
        wc_b = cload("wc_row", [P, D], dt.float32,
                     dram["wc_row"].ap().to_broadcast([P, D]))
        g1_b = cload("g1_row", [P, D], dt.float32,
                     dram["g1_row"].ap().to_broadcast([P, D]))
        g2_b = cload("g2_row", [P, D], dt.float32,
                     dram["g2_row"].ap().to_broadcast([P, D]))
        bc_b = cload("bc_eff", [P, 1], dt.float32,
                     dram["bc_eff"].ap().to_broadcast([P, 1]))
        eps_b = cpool.tile([P, 1], dt.float32, name="eps_b")
        nc.gpsimd.memset(eps_b[:], EPS)

        spatial = dram["spatial"].ap()
        freq = dram["freq"].ap()

        _dma_rr = [0]

        def dma_in(dst, src):
            _dma_rr[0] ^= 1
            (nc.sync if _dma_rr[0] else nc.gpsimd).dma_start(dst, src)

        def transpose_block_group(dst_view, src, blocks, identity, f32r):
            """PE-transpose 4 [128,128] blocks of src into one psum bank, then
            evict to dst_view ([128, 4, 128])."""
            tp = psum("tp")
            for jj, j in enumerate(blocks):
                out_ap = tp[:, jj * P:(jj + 1) * P]
                if f32r:
                    out_ap = out_ap.bitcast(dt.float32r)
                nc.tensor.matmul(out_ap, src[:, j * P:(j + 1) * P], identity[:],
                                 is_transpose=True)
            src_ap = tp[:].rearrange("p (j c) -> p j c", c=P)
            if f32r:
                src_ap = src_ap.bitcast(dt.float32r)
            nc.vector.tensor_copy(dst_view, src_ap)

        def layernorm_from_psum(ps_pair, gamma_b, out_f32r, nm):
            """bn_stats on two psum halves -> (x-mu)*rstd*gamma, f32r row-major.

            normalize runs on ACT (scale=rstd, bias=-mu*rstd per partition);
            gamma multiply on DVE."""
            st = spool.tile([P, 16], dt.float32, tag="st", name=f"st_{nm}")
            nc.vector.bn_stats(st[:, 0:6], ps_pair[0][:])
            nc.vector.bn_stats(st[:, 6:12], ps_pair[1][:])
            nc.vector.bn_aggr(st[:, 12:14], st[:, 0:12])
            sd = spool.tile([P, 4], dt.float32, tag="sd", name=f"sd_{nm}")
            nc.scalar.activation(sd[:, 0:1], st[:, 13:14], Act.Sqrt, bias=eps_b[:])
            nc.vector.reciprocal(sd[:, 1:2], sd[:, 0:1])
            # sd[:,2] = -mu * rstd
            nc.vector.tensor_scalar(sd[:, 2:3], st[:, 12:13], sd[:, 1:2], -1.0,
                                    Alu.mult, Alu.mult)
            for h in range(NH):
                sl = slice(h * 512, (h + 1) * 512)
                nc.scalar.activation(out_f32r[:, sl], ps_pair[h][:], Act.Identity,
                                     bias=sd[:, 2:3], scale=sd[:, 1:2])
                nc.vector.tensor_tensor(out_f32r[:, sl], out_f32r[:, sl],
                                        gamma_b[:, sl], Alu.mult)

        def body(_iv=None):
            for ci in range(NCH):
                r0 = ci * CHUNK
                # ============ F load + transpose -> f_t ============
                f_t = apool.tile([P, DC, CHUNK], dt.float32r, tag="f_t",
                                 name=f"f_t_{ci}")
                for m in range(NM):
                    fr = apool.tile([P, F_DIM], dt.float32r, tag="mb4k", bufs=5,
                                    name=f"fraw_{ci}_{m}")
                    dma_in(fr[:], freq[r0 + m * P: r0 + (m + 1) * P, :])
                    for g in range(2):
                        transpose_block_group(
                            f_t[:, g * 4:g * 4 + 4, m * P:(m + 1) * P],
                            fr[:], range(g * 4, g * 4 + 4), identr, f32r=True)

                # ============ stage a: x_kv feature-major ============
                xkv_t = apool.tile([P, DC, CHUNK], dt.float32r, tag="xkv_t",
                                   name=f"xkv_t_{ci}")
                pa = [psum(f"pa{o}") for o in range(DC)]
                for i in range(DC):
                    wt = apool.tile([P, DC, P], dt.float32r, tag="mb4k", bufs=5,
                                    name=f"wpf_{ci}_{i}")
                    dma_in(wt[:], dram["wpf_t"].ap()[i * P:(i + 1) * P, :]
                           .rearrange("p (o c) -> p o c", c=P))
                    for o in range(DC):
                        nc.tensor.matmul(pa[o][:], wt[:, o, :], f_t[:, i, :],
                                         start=(i == 0), stop=(i == DC - 1))
                for o in range(DC):
                    nc.scalar.activation(xkv_t[:, o, :], pa[o][:], Act.Identity,
                                         bias=ckv_col[:, o:o + 1])

                # ============ S load + transpose -> s_t ============
                s_t = apool.tile([P, SC, CHUNK], dt.float32r, tag="s_t",
                                 name=f"s_t_{ci}")
                for m in range(NM):
                    for half in range(2):
                        sr = apool.tile([P, S_DIM // 2], dt.float32r, tag="mb4k",
                                        bufs=5, name=f"sraw_{ci}_{m}_{half}")
                        dma_in(sr[:], spatial[r0 + m * P: r0 + (m + 1) * P,
                                              half * 1024:(half + 1) * 1024])
                        for g in range(2):
                            transpose_block_group(
                                s_t[:, half * 8 + g * 4: half * 8 + g * 4 + 4,
                                    m * P:(m + 1) * P],
                                sr[:], range(g * 4, g * 4 + 4), identr, f32r=True)

                # ============ stage b/c: t1 = q + attn (+bias) in psum ========
                pbc = [[psum(f"pbc{m}_{h}") for h in range(NH)] for m in range(NM)]
                for i in range(SC):
                    wt = apool.tile([P, 1024], dt.float32r, tag="mb4k", bufs=5,
                                    name=f"wps_{ci}_{i}")
                    dma_in(wt[:], dram["wps_t"].ap()[i * P:(i + 1) * P, :])
                    for m in range(NM):
                        for h in range(NH):
                            nc.tensor.matmul(
                                pbc[m][h][:], s_t[:, i, m * P:(m + 1) * P],
                                wt[:, h * 512:(h + 1) * 512],
                                start=(i == 0), stop=False)
                for i in range(DC):
                    wt = apool.tile([P, 1024], dt.float32r, tag="mb4k", bufs=5,
                                    name=f"wca_{ci}_{i}")
                    dma_in(wt[:], dram["wca_t"].ap()[i * P:(i + 1) * P, :])
                    for m in range(NM):
                        for h in range(NH):
                            nc.tensor.matmul(
                                pbc[m][h][:], xkv_t[:, i, m * P:(m + 1) * P],
                                wt[:, h * 512:(h + 1) * 512],
                                start=False,
                                stop=("c_qca" not in bias_nz and i == DC - 1))
                if "c_qca" in bias_nz:
                    bt = bias_tile("c_qca")
                    for m in range(NM):
                        for h in range(NH):
                            nc.tensor.matmul(
                                pbc[m][h][:], ones_r[:],
                                bt[:, h * 512:(h + 1) * 512],
                                start=False, stop=True)

                # ============ LN1 -> q1s -> q1_t ============
                q1_t = apool.tile([P, DC, CHUNK], dt.float32r, tag="q1_t",
                                  name=f"q1_t_{ci}")
                for m in range(NM):
                    q1s = apool.tile([P, D], dt.float32r, tag="qs", bufs=1,
                                     name=f"q1s_{ci}_{m}")
                    layernorm_from_psum(pbc[m], g1_b, q1s[:], f"ln1_{ci}_{m}")
                    for g in range(2):
                        transpose_block_group(
                            q1_t[:, g * 4:g * 4 + 4, m * P:(m + 1) * P],
                            q1s[:], range(g * 4, g * 4 + 4), identr, f32r=True)

                # ============ stage e: h1g = gelu(q1 @ W1T + b_e) ============
                h1g = apool.tile([P, HC, CHUNK], dt.float32r, tag="h1g",
                                 name=f"h1g_{ci}")
                for o in range(HC):
                    pe = psum(f"pe{o}")
                    wt = apool.tile([P, DC, P], dt.float32r, tag="mb4k", bufs=5,
                                    name=f"w1_{ci}_{o}")
                    dma_in(wt[:], dram["w1_t"].ap()[:, o * P:(o + 1) * P]
                           .rearrange("(i p) c -> p i c", p=P))
                    for i in range(DC):
                        nc.tensor.matmul(pe[:], wt[:, i, :], q1_t[:, i, :],
                                         start=(i == 0), stop=(i == DC - 1))
                    nc.scalar.activation(h1g[:, o, :], pe[:], Act.Gelu,
                                         bias=be_col[:, o:o + 1])

                # ============ stage f: t2 = q1 + mlp_out (+bias) in psum ======
                pf = [[psum(f"pf{m}_{h}") for h in range(NH)] for m in range(NM)]
                for i in range(HC):
                    wt = apool.tile([P, 1024], dt.float32r, tag="mb4k", bufs=5,
                                    name=f"w2_{ci}_{i}")
                    dma_in(wt[:], dram["w2_t"].ap()[i * P:(i + 1) * P, :])
                    for m in range(NM):
                        for h in range(NH):
                            nc.tensor.matmul(
                                pf[m][h][:], h1g[:, i, m * P:(m + 1) * P],
                                wt[:, h * 512:(h + 1) * 512],
                                start=(i == 0), stop=False)
                _fb = "b_f" in bias_nz
                for h in range(NH):
                    for qq in range(2):          # 256-wide output span
                        for bb in range(2):      # two 128-blocks feeding it
                            ii = qq * 2 + bb
                            i = 4 * h + ii
                            for m in range(NM):
                                nc.tensor.matmul(
                                    pf[m][h][:, qq * 256:(qq + 1) * 256],
                                    q1_t[:, i, m * P:(m + 1) * P],
                                    ew[:, 512 - bb * P: 768 - bb * P],
                                    start=False,
                                    stop=(not _fb and qq == 1 and bb == 1))
                if _fb:
                    bt = bias_tile("b_f")
                    for m in range(NM):
                        for h in range(NH):
                            nc.tensor.matmul(
                                pf[m][h][:], ones_r[:],
                                bt[:, h * 512:(h + 1) * 512],
                                start=False, stop=True)

                # ============ LN2 -> q2s -> q2_t ============
                q2_t = apool.tile([P, DC, CHUNK], dt.float32r, tag="q2_t",
                                  name=f"q2_t_{ci}")
                for m in range(NM):
                    q2s = apool.tile([P, D], dt.float32r, tag="qs", bufs=1,
                                     name=f"q2s_{ci}_{m}")
                    layernorm_from_psum(pf[m], g2_b, q2s[:], f"ln2_{ci}_{m}")
                    for g in range(2):
                        transpose_block_group(
                            q2_t[:, g * 4:g * 4 + 4, m * P:(m + 1) * P],
                            q2s[:], range(g * 4, g * 4 + 4), identr, f32r=True)

                # ============ stage g: 2-token self-attention (token 0) =======
                def proj(wname, src_t, bias_name, pfx):
                    pp = [[psum(f"{pfx}{m}_{h}") for h in range(NH)]
                          for m in range(NM)]
                    _hb = bias_name in bias_nz
                    for i in range(DC):
                        wt = apool.tile([P, 1024], dt.float32r, tag="mb4k",
                                        bufs=5, name=f"{pfx}w_{ci}_{i}")
                        dma_in(wt[:], dram[wname].ap()[i * P:(i + 1) * P, :])
                        for m in range(NM):
                            for h in range(NH):
                                nc.tensor.matmul(
                                    pp[m][h][:], src_t[:, i, m * P:(m + 1) * P],
                                    wt[:, h * 512:(h + 1) * 512],
                                    start=(i == 0),
                                    stop=(not _hb and i == DC - 1))
                    if _hb:
                        bt = bias_tile(bias_name)
                        for m in range(NM):
                            for h in range(NH):
                                nc.tensor.matmul(
                                    pp[m][h][:], ones_r[:],
                                    bt[:, h * 512:(h + 1) * 512],
                                    start=False, stop=True)
                    return pp

                dummy = apool.tile([P, P], dt.float32, tag="dummy", bufs=1,
                                   name=f"dummy_{ci}")

                pq0 = proj("wq_t", q2_t, "bq0", "pq0")
                q0s = []
                for m in range(NM):
                    q0 = apool.tile([P, D], dt.float32, tag="sa", bufs=5,
                                    name=f"q0s_{ci}_{m}")
                    for h in range(NH):
                        nc.scalar.activation(q0[:, h * 512:(h + 1) * 512],
                                             pq0[m][h][:], Act.Copy)
                    q0s.append(q0)

                sct = [spool.tile([P, 24], dt.float32, tag="sc", bufs=8,
                                  name=f"sc_{ci}_{m}") for m in range(NM)]

                pk0 = proj("wk_t", q2_t, "bk0", "pk0")
                for m in range(NM):
                    for hd in range(H):
                        nc.vector.scalar_tensor_tensor(
                            dummy[:], q0s[m][:, hd * P:(hd + 1) * P], 1.0,
                            pk0[m][hd // 4][:, (hd % 4) * P:(hd % 4 + 1) * P],
                            Alu.mult, Alu.mult,
                            accum_out=sct[m][:, hd:hd + 1])
                pk1 = proj("wk_t", xkv_t, "bk1", "pk1")
                for m in range(NM):
                    for hd in range(H):
                        nc.vector.scalar_tensor_tensor(
                            dummy[:], q0s[m][:, hd * P:(hd + 1) * P], 1.0,
                            pk1[m][hd // 4][:, (hd % 4) * P:(hd % 4 + 1) * P],
                            Alu.mult, Alu.mult,
                            accum_out=sct[m][:, 8 + hd:9 + hd])
                    nc.vector.tensor_tensor(sct[m][:, 16:24], sct[m][:, 0:8],
                                            sct[m][:, 8:16], Alu.subtract)
                    nc.scalar.activation(sct[m][:, 16:24], sct[m][:, 16:24],
                                         Act.Sigmoid, scale=INV_SQRT_HD)

                pv0 = proj("wv_t", q2_t, "bv0", "pv0")
                v0s = []
                for m in range(NM):
                    v0 = apool.tile([P, D], dt.float32, tag="sa", bufs=5,
                                    name=f"v0s_{ci}_{m}")
                    for h in range(NH):
                        nc.scalar.activation(v0[:, h * 512:(h + 1) * 512],
                                             pv0[m][h][:], Act.Copy)
                    v0s.append(v0)

                pv1 = proj("wv_t", xkv_t, "bv1", "pv1")
                o_t = apool.tile([P, DC, CHUNK], dt.float32r, tag="o_t",
                                 name=f"o_t_{ci}")
                for m in range(NM):
                    e_m = apool.tile([P, D], dt.float32, tag="sa", bufs=5,
                                     name=f"e_{ci}_{m}")
                    for h in range(NH):
                        nc.vector.scalar_tensor_tensor(
                            e_m[:, h * 512:(h + 1) * 512], pv1[m][h][:], 1.0,
                            v0s[m][:, h * 512:(h + 1) * 512],
                            Alu.mult, Alu.subtract)
                    nc.vector.tensor_tensor(
                        e_m[:].rearrange("p (hd c) -> p hd c", c=P),
                        e_m[:].rearrange("p (hd c) -> p hd c", c=P),
                        sct[m][:, 16:24, None].to_broadcast([P, H, P]), Alu.mult)
                    o_m = apool.tile([P, D], dt.float32r, tag="sa", bufs=5,
                                     name=f"o_{ci}_{m}")
                    for h in range(NH):
                        nc.vector.scalar_tensor_tensor(
                            o_m[:, h * 512:(h + 1) * 512], pv1[m][h][:], 1.0,
                            e_m[:, h * 512:(h + 1) * 512],
                            Alu.mult, Alu.subtract)
                    for g in range(2):
                        transpose_block_group(
                            o_t[:, g * 4:g * 4 + 4, m * P:(m + 1) * P],
                            o_m[:], range(g * 4, g * 4 + 4), identr, f32r=True)

                # ============ Wo + residual + bias -> t3; LN3 + classifier ====
                pt3 = [[psum(f"pt3{m}_{h}") for h in range(NH)] for m in range(NM)]
                for i in range(DC):
                    wt = apool.tile([P, 1024], dt.float32r, tag="mb4k", bufs=5,
                                    name=f"wo_{ci}_{i}")
                    dma_in(wt[:], dram["wo_t"].ap()[i * P:(i + 1) * P, :])
                    for m in range(NM):
                        for h in range(NH):
                            nc.tensor.matmul(
                                pt3[m][h][:], o_t[:, i, m * P:(m + 1) * P],
                                wt[:, h * 512:(h + 1) * 512],
                                start=(i == 0), stop=False)
                _gb = "b_g" in bias_nz
                for h in range(NH):
                    for qq in range(2):
                        for bb in range(2):
                            ii = qq * 2 + bb
                            i = 4 * h + ii
                            for m in range(NM):
                                nc.tensor.matmul(
                                    pt3[m][h][:, qq * 256:(qq + 1) * 256],
                                    q2_t[:, i, m * P:(m + 1) * P],
                                    ew[:, 512 - bb * P: 768 - bb * P],
                                    start=False,
                                    stop=(not _gb and qq == 1 and bb == 1))
                if _gb:
                    bt = bias_tile("b_g")
                    for m in range(NM):
                        for h in range(NH):
                            nc.tensor.matmul(
                                pt3[m][h][:], ones_r[:],
                                bt[:, h * 512:(h + 1) * 512],
                                start=False, stop=True)

                outsb = apool.tile([P, NM], dt.float32, tag="outsb", bufs=1,
                                   name=f"outsb_{ci}")
                for m in range(NM):
                    st = spool.tile([P, 16], dt.float32, tag="st",
                                    name=f"st_ln3_{ci}_{m}")
                    nc.vector.bn_stats(st[:, 0:6], pt3[m][0][:])
                    nc.vector.bn_stats(st[:, 6:12], pt3[m][1][:])
                    nc.vector.bn_aggr(st[:, 12:14], st[:, 0:12])
                    sd = spool.tile([P, 4], dt.float32, tag="sd",
                                    name=f"sd_ln3_{ci}_{m}")
                    nc.scalar.activation(sd[:, 0:1], st[:, 13:14], Act.Sqrt,
                                         bias=eps_b[:])
                    nc.vector.reciprocal(sd[:, 1:2], sd[:, 0:1])
                    dsc = apool.tile([P, D], dt.float32, tag="sa", bufs=5,
                                     name=f"dsc_{ci}_{m}")
                    for h in range(NH):
                        nc.vector.scalar_tensor_tensor(
                            dsc[:, h * 512:(h + 1) * 512], pt3[m][h][:],
                            st[:, 12:13], wc_b[:, h * 512:(h + 1) * 512],
                            Alu.subtract, Alu.mult,
                            accum_out=sd[:, 2 + h:3 + h])
                    nc.vector.tensor_tensor(sd[:, 2:3], sd[:, 2:3], sd[:, 3:4],
                                            Alu.add)
                    nc.vector.tensor_tensor(sd[:, 2:3], sd[:, 2:3], sd[:, 1:2],
                                            Alu.mult)
                    nc.vector.tensor_tensor(outsb[:, m:m + 1], sd[:, 2:3],
                                            bc_b[:], Alu.add)
                nc.sync.dma_start(
                    out_d.ap()[r0:r0 + CHUNK, 0].rearrange("(m p) -> p m", p=P),
                    outsb[:])

        if loop_reps > 1:
            with tc.For_i(0, loop_reps, 1) as _i:
                body(_i)
        else:
            body()

    nc.compile()
    return nc


_BUILD_CACHE = {}
_BIAS_NAMES = ("c_qca", "b_f", "bq0", "bk0", "bk1", "bv0", "bv1", "b_g")


def bias_nz_from_consts(consts):
    stack = consts["bias_stack"]
    return tuple(n for i, n in enumerate(_BIAS_NAMES) if np.any(stack[i] != 0))


def _get_nc(loop_reps=1, bias_nz=_BIAS_NAMES):
    key = (loop_reps, tuple(bias_nz))
    if key not in _BUILD_CACHE:
        _BUILD_CACHE[key] = build_kernel(loop_reps, bias_nz)
    return _BUILD_CACHE[key]


def make_in_maps(inputs):
    consts = prep_consts(inputs)
    spatial = _np(inputs["spatial_feat"])
    freq = _np(inputs["freq_feat"])
    in_maps = []
    for c in range(NCORES):
        m = {"spatial": round_f32r(spatial[c * R:(c + 1) * R]),
             "freq": round_f32r(freq[c * R:(c + 1) * R])}
        m.update(consts)
        in_maps.append(m)
    return in_maps


def kernel(**inputs):
    from concourse.bass_utils import run_bass_kernel_spmd
    in_maps = make_in_maps(inputs)
    nc = _get_nc(1, bias_nz_from_consts(in_maps[0]))
    res = run_bass_kernel_spmd(nc, in_maps, core_ids=list(range(NCORES)))
    out = np.concatenate([r["out"] for r in res.results], axis=0)
    return out.astype(np.float32)
